# revision 39
# baseline (speedup 1.0000x reference)
"""BIDE forward kernel for Trainium2, 8-core data parallel over B.

Math: logit[b, v] = sum_h cos(zlo[b, lo(v), h] + zhi[b, hi(v), h]) where
  zlo = bits(lo) @ W[:, :8].T          (lo = v & 255)
  zhi = bits(hi) @ W[:, 8:].T + r      (hi = v >> 8)
Using cos(p+q) = cos p cos q - sin p sin q, the [256, 256] logits table is
two K=128 matmuls over bf16 trig tables of shape [128 h, 256]:
  table = CloT.T @ ChiT - SloT.T @ ShiT   (per batch row)
tbl[v] in DRAM has flat index v = 256*hi + lo = x.

Gather: SWDGE indirect DMA, the only vector-offset DGE path on TRN2.
Its costs are per-descriptor (~1us fixed + 0.34ns/desc gen on Pool,
~2ns/desc through the software queue, 16 completion-sem increments per
call), so descriptors carry FOUR consecutive f32 (16B): offset x>>2 into
tbl viewed [16384, 4]. 4096 descriptors in 4 calls (2 strips per batch
row on SBUF-port-distinct partitions 0/4/16/20) instead of 8192 in 8.
The final 4-way pick (x & 3) plus the strip unpermute and the logZ
subtract happen in the host epilogue.

Range reduction for Sin (only valid on [-pi, pi]): qi = round(q + cp) via
the rounding f32->i32 convert, w = q - qi, with q = z/2pi (weights
pre-scaled by 1/2pi). cp=0 for sin (Sin(2pi w)), cp=0.25 for cos
(Sin(2pi w + pi/2), arg in [-pi, pi]); -sin via scale=-2pi. cos tables
are computed first so the table matmuls' start-pass overlaps the sin
trig.

logZ: Exp(table - 60) with the fused ACT row-sum accumulator; host does
log(sum) + 60.

Each core handles 2 of the 16 batch rows; zero cross-core communication.
"""

import numpy as np
import ml_dtypes
from contextlib import ExitStack

import concourse.bacc as bacc
import concourse.bass as bass
from concourse import mybir
from concourse.bass_utils import run_bass_kernel_spmd
from concourse.tile import TileContext

F32 = mybir.dt.float32
BF16 = mybir.dt.bfloat16
I32 = mybir.dt.int32

PI = float(np.float32(np.pi))
TWO_PI = float(np.float32(2.0 * np.pi))
INV_2PI = 1.0 / (2.0 * np.pi)
# logits for these inputs peak at ~89 (exp overflows fp32); shift exp by a
# constant, added back in the host-side log
EXP_SHIFT = 60.0

N_CORES = 8
B, H, T = 16, 128, 4096
BPC = B // N_CORES  # batch rows per core (2)


def _build():
    nc = bacc.Bacc("TRN2", target_bir_lowering=False, debug=False)

    # lhsT for the z matmuls (cols 0-511), one 128-col group per (b, half):
    # rows 0-7 W_hi bits, 8-15 W_lo residual, 16 r_hi, 17 r_lo (hi half
    # only); cols 512-767 bit-plane enumeration of v in [0, 256)
    wb = nc.dram_tensor("wb", [18, 768], BF16, kind="ExternalInput")
    # gather offsets: col 8*(4b + s) + c holds t = 1024s + 128c + p
    xv = nc.dram_tensor("xv", [128, 64], I32, kind="ExternalInput")
    # row 4b + s = t block [1024s, 1024s + 1024) of batch row b
    outp = nc.dram_tensor("outp", [8, 1024], F32, kind="ExternalOutput")
    sums = nc.dram_tensor("sums", [128, 2], F32, kind="ExternalOutput")

    with ExitStack() as ctx:
        tc = ctx.enter_context(TileContext(nc))
        sb = ctx.enter_context(tc.tile_pool(name="sb", bufs=1))
        ps_z = ctx.enter_context(tc.tile_pool(name="ps_z", bufs=2, space="PSUM"))
        ps_t = ctx.enter_context(tc.tile_pool(name="ps_t", bufs=2, space="PSUM"))
        dram = ctx.enter_context(tc.tile_pool(name="dram", bufs=1, space="DRAM"))

        # ---- input loads
        wb_sb = sb.tile([18, 768], BF16, tag="wb")
        xv_sb = sb.tile([128, 64], I32, tag="xv")
        nc.sync.dma_start(out=wb_sb[:], in_=wb[:])
        nc.sync.dma_start(out=xv_sb[:], in_=xv[:])
        wp_sb = wb_sb[:, 0:512]
        bits_sb = wb_sb[:, 512:768]

        # ---- constants (const-AP registry only has 0/1, so the pi/2 bias
        # needs a per-partition tile)
        pio2 = sb.tile([128, 1], F32, tag="pio2")
        nc.vector.memset(pio2[:], PI / 2.0)
        nshift = sb.tile([128, 1], F32, tag="nshift")
        nc.vector.memset(nshift[:], -EXP_SHIFT)

        # ---- q matmuls: q = z/2pi (weights pre-scaled by 1/2pi), [b0|b1]
        qlo = ps_z.tile([128, 512], F32, tag="z")
        qhi = ps_z.tile([128, 512], F32, tag="z")
        for b in range(BPC):
            bs = slice(256 * b, 256 * b + 256)
            nc.tensor.matmul(
                out=qlo[:, bs],
                lhsT=wp_sb[:, 128 * (2 * b) : 128 * (2 * b) + 128],
                rhs=bits_sb[:],
                start=True,
                stop=True,
            )
            nc.tensor.matmul(
                out=qhi[:, bs],
                lhsT=wp_sb[:, 128 * (2 * b + 1) : 128 * (2 * b + 1) + 128],
                rhs=bits_sb[:],
                start=True,
                stop=True,
            )

        # ---- range reduction + Sin (cos tables first: the table matmuls'
        # start-pass needs only t_a/t_b and overlaps the sin trig)
        def wred(q_ps, cp, tag):
            qi = sb.tile([128, 512], I32, tag=f"qi{tag}")
            if cp == 0.0:
                nc.vector.tensor_copy(out=qi[:], in_=q_ps[:])
            else:
                nc.vector.tensor_scalar(
                    out=qi[:], in0=q_ps[:], scalar1=cp, scalar2=None,
                    op0=mybir.AluOpType.add,
                )
            w = sb.tile([128, 512], F32, tag=f"w{tag}")
            nc.vector.tensor_tensor(
                out=w[:], in0=q_ps[:], in1=qi[:], op=mybir.AluOpType.subtract
            )
            return w

        t_a = sb.tile([128, 512], BF16, tag="t_a")  # cos(zlo)
        t_b = sb.tile([128, 512], BF16, tag="t_b")  # cos(zhi)
        t_c = sb.tile([128, 512], BF16, tag="t_c")  # sin(zlo)
        t_d = sb.tile([128, 512], BF16, tag="t_d")  # -sin(zhi)

        def sin_act(t_t, w_t, scale, bias):
            nc.scalar.activation(
                out=t_t[:], in_=w_t[:],
                func=mybir.ActivationFunctionType.Sin,
                bias=bias if isinstance(bias, float) else bias[:],
                scale=scale,
            )

        sin_act(t_a, wred(qlo, 0.25, "cl"), TWO_PI, pio2)
        sin_act(t_b, wred(qhi, 0.25, "ch"), TWO_PI, pio2)
        sin_act(t_c, wred(qlo, 0.0, "sl"), TWO_PI, 0.0)
        sin_act(t_d, wred(qhi, 0.0, "sh"), -TWO_PI, 0.0)
        t_ps = [
            ps_t.tile([128, 512], F32, tag="tb", name=f"tp{b}")
            for b in range(BPC)
        ]

        # ---- per-b: accumulate -sin(zhi)^T sin(zlo), stage to DRAM,
        # indirect 16B-descriptor gather; exp+sum on ACT in parallel
        sums2 = sb.tile([128, 2], F32, tag="sums2")
        g_sb = sb.tile([32, 1024], F32, tag="g")
        for b in range(BPC):
            bs = slice(256 * b, 256 * b + 256)
            tp = t_ps[b]
            ts = sb.tile([128, 512], F32, tag=f"tsb{b}")
            tbl = dram.tile([65536, 1], F32, tag=f"tbl{b}", name=f"tbl{b}")
            # table[hi, lo] = sum_h cos(zhi)cos(zlo) - sin(zhi)sin(zlo);
            # each start/stop accumulation pair must be adjacent on the PE
            for c in range(2):
                cs = slice(256 * c, 256 * c + 256)
                hi_s = slice(256 * b + 128 * c, 256 * b + 128 * c + 128)
                nc.tensor.matmul(
                    out=tp[:, cs], lhsT=t_b[:, hi_s], rhs=t_a[:, bs],
                    start=True, stop=False,
                )
                nc.tensor.matmul(
                    out=tp[:, cs], lhsT=t_d[:, hi_s], rhs=t_c[:, bs],
                    start=False, stop=True,
                )
                # DMA cannot read PSUM: stage in SBUF, write per c-half so
                # the tbl write overlaps the other half's matmul
                nc.vector.tensor_copy(out=ts[:, cs], in_=tp[:, cs])
                dst = tbl[32768 * c : 32768 * (c + 1), 0:1].rearrange(
                    "(p n) one -> p (n one)", p=128
                )
                nc.sync.dma_start(out=dst, in_=ts[:, cs])
            # 4 strips per row, 1024 single-element descriptors each (the
            # only dest mode the HW DGE honors is [1, N, 1]): offsets walk
            # partition-major (i = 128c + p over the [128, 8] slice), so
            # strip s covers t = 1024s + 128c + p; dest partitions
            # {0,4,8,12} / {16,20,24,28} hit the 8 distinct SBUF write ports
            for s in range(4):
                row = 4 * s + 16 * b
                nc.gpsimd.indirect_dma_start(
                    out=g_sb[row : row + 1, :].rearrange(
                        "one (i x) -> one i x", x=1
                    ),
                    out_offset=None,
                    in_=tbl[:],
                    in_offset=bass.IndirectOffsetOnAxis(
                        ap=xv_sb[:, 32 * b + 8 * s : 32 * b + 8 * s + 8],
                        axis=0,
                    ),
                )
            # partition function: exp with shift (fp32 range) + fused row
            # sums via the ACT accumulator; host does log(sum) + shift
            e = sb.tile([128, 512], BF16, tag=f"e{b}")
            nc.scalar.activation(
                out=e[:], in_=tp[:],
                func=mybir.ActivationFunctionType.Exp, bias=nshift[:],
                accum_out=sums2[:, b : b + 1],
            )

        # ---- outputs (strip partitions {0,4,...,28}, stride-4 AP)
        nc.sync.dma_start(out=outp[:], in_=g_sb[0:32:4, :])
        nc.sync.dma_start(out=sums[:], in_=sums2[:])

    nc.finalize()
    return nc


_NC = None


def _get_nc():
    global _NC
    if _NC is None:
        _NC = _build()
    return _NC


def _bf16_split(a):
    """Return (hi, lo) bf16 arrays with hi + lo ~= a (fp32)."""
    hi = a.astype(ml_dtypes.bfloat16)
    lo = (a - hi.astype(np.float32)).astype(ml_dtypes.bfloat16)
    return hi, lo


def _make_in_maps(x, W, r):
    x = np.asarray(x, dtype=np.int32)
    W = np.asarray(W, dtype=np.float32)
    r = np.asarray(r, dtype=np.float32)

    v = np.arange(256, dtype=np.int32)
    k = np.arange(8, dtype=np.int32)
    bitplanes = ((v[None, :] >> k[:, None]) & 1).astype(np.float32)  # [8, 256]
    bits = np.ones((18, 256), dtype=np.float32)
    bits[0:8] = bitplanes
    bits[8:16] = bitplanes

    in_maps = []
    for core in range(N_CORES):
        wb = np.zeros((18, 768), dtype=ml_dtypes.bfloat16)
        wb[:, 512:768] = bits.astype(ml_dtypes.bfloat16)
        xvs = []
        for b_loc in range(BPC):
            b = BPC * core + b_loc
            for half in range(2):
                g = 2 * b_loc + half
                cs = slice(128 * g, 128 * g + 128)
                w_t = W[b, :, 8 * half : 8 * half + 8].T * INV_2PI  # [8, 128]
                w_hi, w_lo = _bf16_split(w_t.astype(np.float32))
                wb[0:8, cs] = w_hi
                wb[8:16, cs] = w_lo
                if half == 1:
                    r_hi, r_lo = _bf16_split((r[b] * INV_2PI).astype(np.float32))
                    wb[16, cs] = r_hi
                    wb[17, cs] = r_lo
            # xv[p, 8*(4b+s) + c] = x[b, 1024s + 128c + p]
            xvs.append(x[b].reshape(32, 128).T)
        in_maps.append(
            {"wb": wb, "xv": np.concatenate(xvs, axis=1).astype(np.int32)}
        )
    return in_maps


def _run(x, W, r, trace=False):
    nc = _get_nc()
    in_maps = _make_in_maps(x, W, r)
    res = run_bass_kernel_spmd(nc, in_maps, core_ids=list(range(N_CORES)), trace=trace)
    out = np.empty((B, T), dtype=np.float32)
    for core in range(N_CORES):
        g = np.asarray(res.results[core]["outp"], dtype=np.float32)  # [8, 1024]
        s2 = np.asarray(res.results[core]["sums"], dtype=np.float32)  # [128, 2]
        for b_loc in range(BPC):
            b = BPC * core + b_loc
            logz = np.float32(np.log(s2[:, b_loc].sum()) + EXP_SHIFT)
            out[b] = g[4 * b_loc : 4 * b_loc + 4, :].reshape(T) - logz
    return out, res


def kernel(x, W, r):
    out, _ = _run(x, W, r)
    return out


def kernel_traced(x, W, r):
    out, res = _run(x, W, r, trace=True)
    return out, res


# revision 42
# speedup vs baseline: 1.0258x; 1.0258x over previous
"""BIDE forward kernel for Trainium2, 8-core data parallel over B.

Math: logit[b, v] = sum_h cos(zlo[b, lo(v), h] + zhi[b, hi(v), h]) where
  zlo = bits(lo) @ W[:, :8].T          (lo = v & 255)
  zhi = bits(hi) @ W[:, 8:].T + r      (hi = v >> 8)
Using cos(p+q) = cos p cos q - sin p sin q, the [256, 256] logits table is
two K=128 matmuls over bf16 trig tables of shape [128 h, 256]:
  table = CloT.T @ ChiT - SloT.T @ ShiT   (per batch row)
tbl[v] in DRAM has flat index v = 256*hi + lo = x.

Gather: SWDGE indirect DMA, the only vector-offset DGE path on TRN2.
Its costs are per-descriptor (~1us fixed + 0.34ns/desc gen on Pool,
~2ns/desc through the software queue, 16 completion-sem increments per
call), so descriptors carry FOUR consecutive f32 (16B): offset x>>2 into
tbl viewed [16384, 4]. 4096 descriptors in 4 calls (2 strips per batch
row on SBUF-port-distinct partitions 0/4/16/20) instead of 8192 in 8.
The final 4-way pick (x & 3) plus the strip unpermute and the logZ
subtract happen in the host epilogue.

Range reduction for Sin (only valid on [-pi, pi]): qi = round(q + cp) via
the rounding f32->i32 convert, w = q - qi, with q = z/2pi (weights
pre-scaled by 1/2pi). cp=0 for sin (Sin(2pi w)), cp=0.25 for cos
(Sin(2pi w + pi/2), arg in [-pi, pi]); -sin via scale=-2pi. cos tables
are computed first so the table matmuls' start-pass overlaps the sin
trig.

logZ: Exp(table - 60) with the fused ACT row-sum accumulator; host does
log(sum) + 60.

Each core handles 2 of the 16 batch rows; zero cross-core communication.
"""

import numpy as np
import ml_dtypes
from contextlib import ExitStack

import concourse.bacc as bacc
import concourse.bass as bass
from concourse import mybir
from concourse.bass_utils import run_bass_kernel_spmd
from concourse.tile import TileContext

F32 = mybir.dt.float32
BF16 = mybir.dt.bfloat16
I32 = mybir.dt.int32

PI = float(np.float32(np.pi))
TWO_PI = float(np.float32(2.0 * np.pi))
INV_2PI = 1.0 / (2.0 * np.pi)
# logits for these inputs peak at ~89 (exp overflows fp32); shift exp by a
# constant, added back in the host-side log
EXP_SHIFT = 60.0

N_CORES = 8
B, H, T = 16, 128, 4096
BPC = B // N_CORES  # batch rows per core (2)


def _build():
    nc = bacc.Bacc("TRN2", target_bir_lowering=False, debug=False)

    # lhsT for the z matmuls (cols 0-511), one 128-col group per (b, half):
    # rows 0-7 W_hi bits, 8-15 W_lo residual, 16 r_hi, 17 r_lo (hi half
    # only); cols 512-767 bit-plane enumeration of v in [0, 256)
    wb = nc.dram_tensor("wb", [18, 768], BF16, kind="ExternalInput")
    # gather offsets: col 8*(4b + s) + c holds t = 1024s + 128c + p
    xv = nc.dram_tensor("xv", [128, 64], I32, kind="ExternalInput")
    # row 4b + s = t block [1024s, 1024s + 1024) of batch row b
    outp = nc.dram_tensor("outp", [8, 1024], F32, kind="ExternalOutput")
    sums = nc.dram_tensor("sums", [128, 2], F32, kind="ExternalOutput")

    with ExitStack() as ctx:
        tc = ctx.enter_context(TileContext(nc))
        sb = ctx.enter_context(tc.tile_pool(name="sb", bufs=1))
        ps_z = ctx.enter_context(tc.tile_pool(name="ps_z", bufs=2, space="PSUM"))
        ps_t = ctx.enter_context(tc.tile_pool(name="ps_t", bufs=2, space="PSUM"))
        dram = ctx.enter_context(tc.tile_pool(name="dram", bufs=1, space="DRAM"))

        # ---- input loads
        wb_sb = sb.tile([18, 768], BF16, tag="wb")
        xv_sb = sb.tile([128, 64], I32, tag="xv")
        nc.sync.dma_start(out=wb_sb[:], in_=wb[:])
        nc.sync.dma_start(out=xv_sb[:], in_=xv[:])
        wp_sb = wb_sb[:, 0:512]
        bits_sb = wb_sb[:, 512:768]

        # ---- constants (const-AP registry only has 0/1, so the pi/2 bias
        # needs a per-partition tile)
        pio2 = sb.tile([128, 1], F32, tag="pio2")
        nc.vector.memset(pio2[:], PI / 2.0)
        nshift = sb.tile([128, 1], F32, tag="nshift")
        nc.vector.memset(nshift[:], -EXP_SHIFT)

        # ---- q matmuls: q = z/2pi (weights pre-scaled by 1/2pi), [b0|b1]
        qlo = ps_z.tile([128, 512], F32, tag="z")
        qhi = ps_z.tile([128, 512], F32, tag="z")
        for b in range(BPC):
            bs = slice(256 * b, 256 * b + 256)
            nc.tensor.matmul(
                out=qlo[:, bs],
                lhsT=wp_sb[:, 128 * (2 * b) : 128 * (2 * b) + 128],
                rhs=bits_sb[:],
                start=True,
                stop=True,
            )
            nc.tensor.matmul(
                out=qhi[:, bs],
                lhsT=wp_sb[:, 128 * (2 * b + 1) : 128 * (2 * b + 1) + 128],
                rhs=bits_sb[:],
                start=True,
                stop=True,
            )

        # ---- range reduction + Sin, per batch row ([128, 256] halves) so
        # b0's table chain (and its gather descriptor-gen on Pool) starts
        # while b1's trig still runs
        def wred(q_ps, bs, cp, tag):
            qi = sb.tile([128, 256], I32, tag=f"qi{tag}", name=f"qi{tag}")
            if cp == 0.0:
                nc.vector.tensor_copy(out=qi[:], in_=q_ps[:, bs])
            else:
                nc.vector.tensor_scalar(
                    out=qi[:], in0=q_ps[:, bs], scalar1=cp, scalar2=None,
                    op0=mybir.AluOpType.add,
                )
            w = sb.tile([128, 256], F32, tag=f"w{tag}", name=f"w{tag}")
            nc.vector.tensor_tensor(
                out=w[:], in0=q_ps[:, bs], in1=qi[:],
                op=mybir.AluOpType.subtract,
            )
            return w

        t_a = sb.tile([128, 512], BF16, tag="t_a")  # cos(zlo)
        t_b = sb.tile([128, 512], BF16, tag="t_b")  # cos(zhi)
        t_c = sb.tile([128, 512], BF16, tag="t_c")  # sin(zlo)
        t_d = sb.tile([128, 512], BF16, tag="t_d")  # -sin(zhi)

        def sin_act(t_t, bs, w_t, scale, bias):
            nc.scalar.activation(
                out=t_t[:, bs], in_=w_t[:],
                func=mybir.ActivationFunctionType.Sin,
                bias=bias if isinstance(bias, float) else bias[:],
                scale=scale,
            )

        def trig_for_b(b):
            bs = slice(256 * b, 256 * b + 256)
            sin_act(t_a, bs, wred(qlo, bs, 0.25, f"cl{b}"), TWO_PI, pio2)
            sin_act(t_b, bs, wred(qhi, bs, 0.25, f"ch{b}"), TWO_PI, pio2)
            sin_act(t_c, bs, wred(qlo, bs, 0.0, f"sl{b}"), TWO_PI, 0.0)
            sin_act(t_d, bs, wred(qhi, bs, 0.0, f"sh{b}"), -TWO_PI, 0.0)

        t_ps = [
            ps_t.tile([128, 512], F32, tag="tb", name=f"tp{b}")
            for b in range(BPC)
        ]

        # ---- per-b: accumulate -sin(zhi)^T sin(zlo), stage to DRAM,
        # indirect 16B-descriptor gather; exp+sum on ACT in parallel
        sums2 = sb.tile([128, 2], F32, tag="sums2")
        g_sb = sb.tile([32, 1024], F32, tag="g")
        for b in range(BPC):
            trig_for_b(b)
            bs = slice(256 * b, 256 * b + 256)
            tp = t_ps[b]
            ts = sb.tile([128, 512], F32, tag=f"tsb{b}")
            tbl = dram.tile([65536, 1], F32, tag=f"tbl{b}", name=f"tbl{b}")
            # table[hi, lo] = sum_h cos(zhi)cos(zlo) - sin(zhi)sin(zlo);
            # each start/stop accumulation pair must be adjacent on the PE
            for c in range(2):
                cs = slice(256 * c, 256 * c + 256)
                hi_s = slice(256 * b + 128 * c, 256 * b + 128 * c + 128)
                nc.tensor.matmul(
                    out=tp[:, cs], lhsT=t_b[:, hi_s], rhs=t_a[:, bs],
                    start=True, stop=False,
                )
                nc.tensor.matmul(
                    out=tp[:, cs], lhsT=t_d[:, hi_s], rhs=t_c[:, bs],
                    start=False, stop=True,
                )
                # DMA cannot read PSUM: stage in SBUF, write per c-half so
                # the tbl write overlaps the other half's matmul
                nc.vector.tensor_copy(out=ts[:, cs], in_=tp[:, cs])
                dst = tbl[32768 * c : 32768 * (c + 1), 0:1].rearrange(
                    "(p n) one -> p (n one)", p=128
                )
                nc.sync.dma_start(out=dst, in_=ts[:, cs])
            # 4 strips per row, 1024 single-element descriptors each (the
            # only dest mode the HW DGE honors is [1, N, 1]): offsets walk
            # partition-major (i = 128c + p over the [128, 8] slice), so
            # strip s covers t = 1024s + 128c + p; dest partitions
            # {0,4,8,12} / {16,20,24,28} hit the 8 distinct SBUF write ports
            for s in range(4):
                row = 4 * s + 16 * b
                nc.gpsimd.indirect_dma_start(
                    out=g_sb[row : row + 1, :].rearrange(
                        "one (i x) -> one i x", x=1
                    ),
                    out_offset=None,
                    in_=tbl[:],
                    in_offset=bass.IndirectOffsetOnAxis(
                        ap=xv_sb[:, 32 * b + 8 * s : 32 * b + 8 * s + 8],
                        axis=0,
                    ),
                )
        # ---- partition function after both Sin blocks (one ACT table
        # switch): exp with shift (fp32 range) + fused row sums via the
        # ACT accumulator; host does log(sum) + shift
        for b in range(BPC):
            e = sb.tile([128, 512], BF16, tag=f"e{b}", name=f"e{b}")
            nc.scalar.activation(
                out=e[:], in_=t_ps[b][:],
                func=mybir.ActivationFunctionType.Exp, bias=nshift[:],
                accum_out=sums2[:, b : b + 1],
            )

        # ---- outputs (strip partitions {0,4,...,28}, stride-4 AP)
        nc.sync.dma_start(out=outp[:], in_=g_sb[0:32:4, :])
        nc.sync.dma_start(out=sums[:], in_=sums2[:])

    nc.finalize()
    return nc


_NC = None


def _get_nc():
    global _NC
    if _NC is None:
        _NC = _build()
    return _NC


def _bf16_split(a):
    """Return (hi, lo) bf16 arrays with hi + lo ~= a (fp32)."""
    hi = a.astype(ml_dtypes.bfloat16)
    lo = (a - hi.astype(np.float32)).astype(ml_dtypes.bfloat16)
    return hi, lo


def _make_in_maps(x, W, r):
    x = np.asarray(x, dtype=np.int32)
    W = np.asarray(W, dtype=np.float32)
    r = np.asarray(r, dtype=np.float32)

    v = np.arange(256, dtype=np.int32)
    k = np.arange(8, dtype=np.int32)
    bitplanes = ((v[None, :] >> k[:, None]) & 1).astype(np.float32)  # [8, 256]
    bits = np.ones((18, 256), dtype=np.float32)
    bits[0:8] = bitplanes
    bits[8:16] = bitplanes

    in_maps = []
    for core in range(N_CORES):
        wb = np.zeros((18, 768), dtype=ml_dtypes.bfloat16)
        wb[:, 512:768] = bits.astype(ml_dtypes.bfloat16)
        xvs = []
        for b_loc in range(BPC):
            b = BPC * core + b_loc
            for half in range(2):
                g = 2 * b_loc + half
                cs = slice(128 * g, 128 * g + 128)
                w_t = W[b, :, 8 * half : 8 * half + 8].T * INV_2PI  # [8, 128]
                w_hi, w_lo = _bf16_split(w_t.astype(np.float32))
                wb[0:8, cs] = w_hi
                wb[8:16, cs] = w_lo
                if half == 1:
                    r_hi, r_lo = _bf16_split((r[b] * INV_2PI).astype(np.float32))
                    wb[16, cs] = r_hi
                    wb[17, cs] = r_lo
            # xv[p, 8*(4b+s) + c] = x[b, 1024s + 128c + p]
            xvs.append(x[b].reshape(32, 128).T)
        in_maps.append(
            {"wb": wb, "xv": np.concatenate(xvs, axis=1).astype(np.int32)}
        )
    return in_maps


def _run(x, W, r, trace=False):
    nc = _get_nc()
    in_maps = _make_in_maps(x, W, r)
    res = run_bass_kernel_spmd(nc, in_maps, core_ids=list(range(N_CORES)), trace=trace)
    out = np.empty((B, T), dtype=np.float32)
    for core in range(N_CORES):
        g = np.asarray(res.results[core]["outp"], dtype=np.float32)  # [8, 1024]
        s2 = np.asarray(res.results[core]["sums"], dtype=np.float32)  # [128, 2]
        for b_loc in range(BPC):
            b = BPC * core + b_loc
            logz = np.float32(np.log(s2[:, b_loc].sum()) + EXP_SHIFT)
            out[b] = g[4 * b_loc : 4 * b_loc + 4, :].reshape(T) - logz
    return out, res


def kernel(x, W, r):
    out, _ = _run(x, W, r)
    return out


def kernel_traced(x, W, r):
    out, res = _run(x, W, r, trace=True)
    return out, res
